# revision 3
# baseline (speedup 1.0000x reference)
"""AFT-Full transformer encoder block on 8 Trainium2 NeuronCores.

Sharding: data-parallel over batch (B=8 -> 1 batch element per core), all
weights replicated. No collectives.

Per-core layout strategy ("T-layout"): every on-chip activation that feeds a
matmul keeps its contraction dimension on SBUF partitions, so the kernel
needs zero on-chip transposes:
  - host feeds x[b] transposed:  xT [F=512, T=1024]
  - host feeds w_pos transposed: w_posT [S=1024, T=1024] (bf16)
  - hT [f,t]: lhsT for K/V (N-layout out [t,f]) and rhs for Q (T-layout out)
  - X = [exp_K*V | exp_K] in N-layout [s, 2F] is the lhsT of num/den
  - exp_wT [s,t] is the rhs of num/den -> numT/denT [f,t] (T-layout)
  - attn/mlp outputs stay T-layout; output yT [F, T] transposed back on host

Matmuls run in bf16 (1 cycle/row, fp32 PSUM accumulation); layernorm
statistics, reciprocals, residuals are fp32.  The exp_w row-max subtraction
is skipped: it cancels exactly in num/den.  LayerNorm statistics come from
an all-ones [128,128] stationary matmul, which yields partition-replicated
sums (no partition broadcast needed).  Squares / casts / final residual adds
run on the otherwise-idle GPSIMD engine.

Two program variants: the "trivial" one assumes ln gains==1, ln biases==0 and
all linear biases==0 (exactly what this problem's setup_inputs produces) and
skips the corresponding work; the general one implements them faithfully.
kernel() inspects the inputs and dispatches.
"""
import functools
import numpy as np
import ml_dtypes

import concourse.bacc as bacc
import concourse.tile as tile
import concourse.mybir as mybir
from concourse.bass_utils import run_bass_kernel_spmd

P = 128
B, T, F, H = 8, 1024, 512, 2048
FT = F // P      # 4 feature tiles
TT = T // P      # 8 token tiles
HT = H // P      # 16 hidden tiles
CH = 512         # token chunk (one PSUM bank of fp32)
NC = T // CH     # 2 chunks
LN_EPS = 1e-5

f32 = mybir.dt.float32
bf16 = mybir.dt.bfloat16
ALU = mybir.AluOpType
AF = mybir.ActivationFunctionType


def _emit_layernorm(nc, tc, psum, ln_tmp, src, srcb, sqb, g_pm, b_pm, ones,
                    out_b, trivial):
    """LayerNorm over the partition (feature) axis of T-layout src.

    src fp32 / srcb bf16 / sqb bf16: [P, FT, T];  out_b: bf16 [P, FT, T]
    Stats = ones[128,128].T @ {srcb, sqb} -> partition-replicated column sums.
    If trivial, gain/bias application is skipped (gain==1, bias==0).
    """
    mval = ln_tmp.tile([P, T], f32, tag="mval")
    var = ln_tmp.tile([P, T], f32, tag="var")
    for c in range(NC):
        ts = slice(c * CH, (c + 1) * CH)
        s1 = psum.tile([P, CH], f32, tag="acc")
        for ft in range(FT):
            nc.tensor.matmul(s1[:], ones[:, :P], srcb[:, ft, ts],
                             start=(ft == 0), stop=(ft == FT - 1))
        s2 = psum.tile([P, CH], f32, tag="acc")
        for ft in range(FT):
            nc.tensor.matmul(s2[:], ones[:, :P], sqb[:, ft, ts],
                             start=(ft == 0), stop=(ft == FT - 1))
        nc.vector.tensor_scalar_mul(mval[:, ts], s1[:], 1.0 / F)
        msq = ln_tmp.tile([P, CH], f32, tag="msq")
        nc.vector.tensor_tensor(msq[:], mval[:, ts], mval[:, ts], op=ALU.mult)
        nc.vector.scalar_tensor_tensor(var[:, ts], s2[:], 1.0 / F, msq[:],
                                       op0=ALU.mult, op1=ALU.subtract)
    varp = ln_tmp.tile([P, T], f32, tag="varp")
    nc.vector.tensor_scalar_add(varp[:], var[:], LN_EPS)
    rcv = ln_tmp.tile([P, T], f32, tag="rcv")
    nc.vector.reciprocal(rcv[:], varp[:])
    rstd = ln_tmp.tile([P, T], bf16, tag="rstd")
    nc.scalar.activation(rstd[:], rcv[:], AF.Sqrt)
    rm = ln_tmp.tile([P, T], bf16, tag="rm")
    nc.vector.tensor_tensor(rm[:], rstd[:], mval[:], op=ALU.mult)
    for ft in range(FT):
        if trivial:
            t0 = ln_tmp.tile([P, T], bf16, tag="t0")
            nc.vector.tensor_tensor(t0[:], srcb[:, ft, :], rstd[:], op=ALU.mult)
            nc.vector.tensor_tensor(out_b[:, ft, :], t0[:], rm[:],
                                    op=ALU.subtract)
        else:
            t0 = ln_tmp.tile([P, T], bf16, tag="t0")
            nc.vector.tensor_tensor(t0[:], srcb[:, ft, :], rstd[:], op=ALU.mult)
            t1 = ln_tmp.tile([P, T], bf16, tag="t1")
            nc.vector.tensor_tensor(t1[:], t0[:], rm[:], op=ALU.subtract)
            nc.scalar.activation(out_b[:, ft, :], t1[:], AF.Identity,
                                 bias=b_pm[:, ft:ft + 1], scale=g_pm[:, ft:ft + 1])


def build_nc(trivial):
    nc = bacc.Bacc("TRN2", target_bir_lowering=False)

    xT_d = nc.dram_tensor("xT", (F, T), f32, kind="ExternalInput")
    wposT_d = nc.dram_tensor("w_posT", (T, T), bf16, kind="ExternalInput")
    wq_d = nc.dram_tensor("wq", (F, F), bf16, kind="ExternalInput")
    wk_d = nc.dram_tensor("wk", (F, F), bf16, kind="ExternalInput")
    wv_d = nc.dram_tensor("wv", (F, F), bf16, kind="ExternalInput")
    ow_d = nc.dram_tensor("ow", (F, F), bf16, kind="ExternalInput")
    w1_d = nc.dram_tensor("w1", (F, H), bf16, kind="ExternalInput")
    w2_d = nc.dram_tensor("w2", (H, F), bf16, kind="ExternalInput")
    wqb_d = nc.dram_tensor("wq_b", (F,), f32, kind="ExternalInput")
    wkb_d = nc.dram_tensor("wk_b", (F,), bf16, kind="ExternalInput")
    wvb_d = nc.dram_tensor("wv_b", (F,), bf16, kind="ExternalInput")
    outb_d = nc.dram_tensor("out_b", (F,), bf16, kind="ExternalInput")
    ln1g_d = nc.dram_tensor("ln1_g", (F,), f32, kind="ExternalInput")
    ln1b_d = nc.dram_tensor("ln1_b", (F,), f32, kind="ExternalInput")
    ln2g_d = nc.dram_tensor("ln2_g", (F,), f32, kind="ExternalInput")
    ln2b_d = nc.dram_tensor("ln2_b", (F,), f32, kind="ExternalInput")
    b1_d = nc.dram_tensor("mlp1_b", (H,), f32, kind="ExternalInput")
    b2_d = nc.dram_tensor("mlp2_b", (F,), f32, kind="ExternalInput")
    yT_d = nc.dram_tensor("yT", (F, T), f32, kind="ExternalOutput")

    with tile.TileContext(nc) as tc:
        with (
            tc.tile_pool(name="persist", bufs=1) as pp,
            tc.tile_pool(name="ln_tmp", bufs=2) as ln_tmp,
            tc.tile_pool(name="outstream", bufs=3) as outp,
            tc.tile_pool(name="psum", bufs=4, space="PSUM") as psum,
            tc.tile_pool(name="psum2", bufs=2, space="PSUM") as psum2,
        ):
            # ---- persistent loads / constants
            xT = pp.tile([P, FT, T], f32)
            for ft in range(FT):
                nc.sync.dma_start(xT[:, ft, :], xT_d[ft * P:(ft + 1) * P, :])
            wq = pp.tile([P, FT, F], bf16)
            nc.sync.dma_start(wq[:], wq_d.rearrange("(a p) b -> p a b", p=P))
            wk = pp.tile([P, FT, F], bf16)
            nc.sync.dma_start(wk[:], wk_d.rearrange("(a p) b -> p a b", p=P))
            wv = pp.tile([P, FT, F], bf16)
            nc.sync.dma_start(wv[:], wv_d.rearrange("(a p) b -> p a b", p=P))
            ow = pp.tile([P, FT, F], bf16)
            nc.sync.dma_start(ow[:], ow_d.rearrange("(a p) b -> p a b", p=P))
            ones = pp.tile([P, T], bf16)
            nc.vector.memset(ones[:], 1.0)
            if not trivial:
                wqb = pp.tile([P, FT], f32)
                nc.sync.dma_start(wqb[:], wqb_d.rearrange("(a p) -> p a", p=P))
                wkb = pp.tile([1, F], bf16)
                nc.sync.dma_start(wkb[:], wkb_d[None, :])
                wvb = pp.tile([1, F], bf16)
                nc.sync.dma_start(wvb[:], wvb_d[None, :])
                outb = pp.tile([1, F], bf16)
                nc.sync.dma_start(outb[:], outb_d[None, :])
                ln1g = pp.tile([P, FT], f32)
                nc.sync.dma_start(ln1g[:], ln1g_d.rearrange("(a p) -> p a", p=P))
                ln1b = pp.tile([P, FT], f32)
                nc.sync.dma_start(ln1b[:], ln1b_d.rearrange("(a p) -> p a", p=P))
                ln2g = pp.tile([P, FT], f32)
                nc.sync.dma_start(ln2g[:], ln2g_d.rearrange("(a p) -> p a", p=P))
                ln2b = pp.tile([P, FT], f32)
                nc.sync.dma_start(ln2b[:], ln2b_d.rearrange("(a p) -> p a", p=P))
                b1 = pp.tile([P, HT], f32)
                nc.sync.dma_start(b1[:], b1_d.rearrange("(a p) -> p a", p=P))
                b2 = pp.tile([P, FT], f32)
                nc.sync.dma_start(b2[:], b2_d.rearrange("(a p) -> p a", p=P))
            else:
                wqb = wkb = wvb = outb = None
                ln1g = ln1b = ln2g = ln2b = b1 = b2 = None

            yt = pp.tile([P, FT, T], bf16)    # sigma(Q)*num/den, T-layout
            outT = pp.tile([P, FT, T], f32)   # attn residual output, T-layout

            with tc.tile_pool(name="phaseA", bufs=1) as pa:
                xb = pa.tile([P, FT, T], bf16)
                sqb = pa.tile([P, FT, T], bf16)
                for ft in range(FT):
                    nc.gpsimd.tensor_copy(xb[:, ft, :], xT[:, ft, :])
                    nc.gpsimd.tensor_tensor(sqb[:, ft, :], xT[:, ft, :],
                                            xT[:, ft, :], op=ALU.mult)

                # ---- LN1 -> hTb (bf16, T-layout)
                hTb = pa.tile([P, FT, T], bf16)
                _emit_layernorm(nc, tc, psum, ln_tmp, xT, xb, sqb,
                                ln1g, ln1b, ones, hTb, trivial)

                # ---- exp_wT (bf16): stream w_posT tiles, exp on ACT
                expw = pa.tile([P, TT, T], bf16)
                with tc.tile_pool(name="wpos", bufs=2) as wpp:
                    for s in range(TT):
                        wp = wpp.tile([P, T], bf16, tag="wp")
                        nc.sync.dma_start(wp[:], wposT_d[s * P:(s + 1) * P, :])
                        nc.scalar.activation(expw[:, s, :], wp[:], AF.Exp)

                # ---- K, V (N-layout [t, fo]) -> X = [ekV | ek] bf16 [P, TT, 2F]
                X = pa.tile([P, TT, 2 * F], bf16)
                for s in range(TT):
                    tsl = slice(s * P, (s + 1) * P)
                    kps = psum.tile([P, F], f32, tag="acc")
                    for ft in range(FT):
                        nc.tensor.matmul(kps[:], hTb[:, ft, tsl], wk[:, ft, :],
                                         start=(ft == 0),
                                         stop=(ft == FT - 1 and trivial))
                    if not trivial:
                        nc.tensor.matmul(kps[:], ones[0:1, :P], wkb[:],
                                         start=False, stop=True)
                    negmk = ln_tmp.tile([P, 1], f32, tag="negmk")
                    nc.vector.tensor_reduce(negmk[:], kps[:],
                                            axis=mybir.AxisListType.X,
                                            op=ALU.max, negate=True)
                    nc.scalar.activation(X[:, s, F:], kps[:], AF.Exp,
                                         bias=negmk[:], scale=1.0)
                    vps = psum.tile([P, F], f32, tag="acc")
                    for ft in range(FT):
                        nc.tensor.matmul(vps[:], hTb[:, ft, tsl], wv[:, ft, :],
                                         start=(ft == 0),
                                         stop=(ft == FT - 1 and trivial))
                    if not trivial:
                        nc.tensor.matmul(vps[:], ones[0:1, :P], wvb[:],
                                         start=False, stop=True)
                    nc.vector.tensor_tensor(X[:, s, :F], X[:, s, F:], vps[:],
                                            op=ALU.mult)

                # ---- Q (T-layout) -> sigQ (bf16)
                sigq = pa.tile([P, FT, T], bf16)
                for fo in range(FT):
                    for c in range(NC):
                        ts = slice(c * CH, (c + 1) * CH)
                        qps = psum.tile([P, CH], f32, tag="acc")
                        for ft in range(FT):
                            nc.tensor.matmul(qps[:],
                                             wq[:, ft, fo * P:(fo + 1) * P],
                                             hTb[:, ft, ts],
                                             start=(ft == 0), stop=(ft == FT - 1))
                        bias = 0.0 if trivial else wqb[:, fo:fo + 1]
                        nc.scalar.activation(sigq[:, fo, ts], qps[:], AF.Sigmoid,
                                             bias=bias, scale=1.0)

                # ---- num/den:  numT/denT[f, t] = X.T @ exp_wT  -> Yt
                with tc.tile_pool(name="ndtmp", bufs=2) as ndt:
                    for fo in range(FT):
                        for c in range(NC):
                            ts = slice(c * CH, (c + 1) * CH)
                            dps = psum.tile([P, CH], f32, tag="acc")
                            for s in range(TT):
                                nc.tensor.matmul(
                                    dps[:],
                                    X[:, s, F + fo * P:F + (fo + 1) * P],
                                    expw[:, s, ts],
                                    start=(s == 0), stop=(s == TT - 1))
                            rcden = ndt.tile([P, CH], f32, tag="rcden")
                            nc.vector.reciprocal(rcden[:], dps[:])
                            nps = psum.tile([P, CH], f32, tag="acc")
                            for s in range(TT):
                                nc.tensor.matmul(
                                    nps[:],
                                    X[:, s, fo * P:(fo + 1) * P],
                                    expw[:, s, ts],
                                    start=(s == 0), stop=(s == TT - 1))
                            t1 = ndt.tile([P, CH], bf16, tag="t1")
                            nc.vector.tensor_tensor(t1[:], nps[:], rcden[:],
                                                    op=ALU.mult)
                            nc.vector.tensor_tensor(yt[:, fo, ts], t1[:],
                                                    sigq[:, fo, ts], op=ALU.mult)

            # ---- attn out (T-layout) + residual: outT = ow.T @ Yt (+ out_b) + xT
            for g in range(FT):
                for c in range(NC):
                    ts = slice(c * CH, (c + 1) * CH)
                    aps = psum.tile([P, CH], f32, tag="acc")
                    for ft in range(FT):
                        nc.tensor.matmul(aps[:], ow[:, ft, g * P:(g + 1) * P],
                                         yt[:, ft, ts],
                                         start=(ft == 0),
                                         stop=(ft == FT - 1 and trivial))
                    if not trivial:
                        nc.tensor.matmul(aps[:], outb[0:1, g * P:(g + 1) * P],
                                         ones[0:1, :CH], start=False, stop=True)
                    nc.vector.scalar_tensor_tensor(outT[:, g, ts], aps[:], 1.0,
                                                   xT[:, g, ts],
                                                   op0=ALU.mult, op1=ALU.add)

            with tc.tile_pool(name="phaseB", bufs=1) as pb:
                # ---- LN2 -> mTb
                outb16 = pb.tile([P, FT, T], bf16)
                sq2b = pb.tile([P, FT, T], bf16)
                for ft in range(FT):
                    nc.gpsimd.tensor_copy(outb16[:, ft, :], outT[:, ft, :])
                    nc.gpsimd.tensor_tensor(sq2b[:, ft, :], outT[:, ft, :],
                                            outT[:, ft, :], op=ALU.mult)
                mTb = pb.tile([P, FT, T], bf16)
                _emit_layernorm(nc, tc, psum, ln_tmp, outT, outb16, sq2b,
                                ln2g, ln2b, ones, mTb, trivial)

                # ---- MLP
                w1 = pb.tile([P, FT, H], bf16)
                for ft in range(FT):
                    nc.sync.dma_start(
                        w1[:, ft, :], w1_d[ft * P:(ft + 1) * P, :])
                w2 = pb.tile([P, HT, F], bf16)
                for ht in range(HT):
                    nc.sync.dma_start(
                        w2[:, ht, :], w2_d[ht * P:(ht + 1) * P, :])

                m1 = pb.tile([P, HT, T], bf16)
                for ht in range(HT):
                    mps = psum2.tile([P, T], f32, tag="acc2")
                    for c in range(NC):
                        ts = slice(c * CH, (c + 1) * CH)
                        for ft in range(FT):
                            nc.tensor.matmul(mps[:, ts],
                                             w1[:, ft, ht * P:(ht + 1) * P],
                                             mTb[:, ft, ts],
                                             start=(ft == 0), stop=(ft == FT - 1))
                    bias = 0.0 if trivial else b1[:, ht:ht + 1]
                    nc.scalar.activation(m1[:, ht, :], mps[:], AF.Gelu,
                                         bias=bias, scale=1.0)

                for g in range(FT):
                    for c in range(NC):
                        ts = slice(c * CH, (c + 1) * CH)
                        fps = psum.tile([P, CH], f32, tag="acc")
                        for ht in range(HT):
                            nc.tensor.matmul(fps[:],
                                             w2[:, ht, g * P:(g + 1) * P],
                                             m1[:, ht, ts],
                                             start=(ht == 0), stop=(ht == HT - 1))
                        gt = outp.tile([P, CH], f32, tag="gt")
                        bias = 0.0 if trivial else b2[:, g:g + 1]
                        nc.scalar.activation(gt[:], fps[:], AF.Gelu,
                                             bias=bias, scale=1.0)
                        fin = outp.tile([P, CH], f32, tag="fin")
                        nc.gpsimd.tensor_tensor(fin[:], gt[:], outT[:, g, ts],
                                                op=ALU.add)
                        nc.sync.dma_start(yT_d[g * P:(g + 1) * P, ts], fin[:])
    nc.compile()
    return nc


@functools.lru_cache(maxsize=2)
def _get_nc(trivial=True):
    return build_nc(trivial)


def _is_trivial(inputs):
    z = lambda k: not np.any(np.asarray(inputs[k]))
    o = lambda k: np.all(np.asarray(inputs[k]) == 1.0)
    return (z("wq_b") and z("wk_b") and z("wv_b") and z("out_b")
            and z("mlp1_b") and z("mlp2_b") and z("ln1_b") and z("ln2_b")
            and o("ln1_g") and o("ln2_g"))


def make_in_maps(inputs):
    x = np.asarray(inputs["x"], dtype=np.float32)
    bf = lambda a: np.ascontiguousarray(np.asarray(a)).astype(ml_dtypes.bfloat16)
    fl = lambda a: np.ascontiguousarray(np.asarray(a), dtype=np.float32)
    shared = {
        "w_posT": bf(np.asarray(inputs["w_pos"]).T),
        "wq": bf(inputs["wq_w"]), "wk": bf(inputs["wk_w"]),
        "wv": bf(inputs["wv_w"]), "ow": bf(inputs["out_w"]),
        "w1": bf(inputs["mlp1_w"]), "w2": bf(inputs["mlp2_w"]),
        "wq_b": fl(inputs["wq_b"]), "wk_b": bf(inputs["wk_b"]),
        "wv_b": bf(inputs["wv_b"]), "out_b": bf(inputs["out_b"]),
        "ln1_g": fl(inputs["ln1_g"]), "ln1_b": fl(inputs["ln1_b"]),
        "ln2_g": fl(inputs["ln2_g"]), "ln2_b": fl(inputs["ln2_b"]),
        "mlp1_b": fl(inputs["mlp1_b"]), "mlp2_b": fl(inputs["mlp2_b"]),
    }
    return [{"xT": np.ascontiguousarray(x[c].T), **shared} for c in range(B)]


def kernel(**inputs):
    nc = _get_nc(_is_trivial(inputs))
    res = run_bass_kernel_spmd(nc, make_in_maps(inputs), list(range(B)))
    out = np.stack([np.ascontiguousarray(res.results[c]["yT"].T)
                    for c in range(B)], axis=0)
    return out.astype(np.float32)


if __name__ == "__main__":
    rng = np.random.default_rng(0)
    fake = {
        "x": rng.standard_normal((B, T, F), dtype=np.float32),
        "wq_w": rng.standard_normal((F, F), dtype=np.float32) * 0.02,
        "wq_b": np.zeros(F, np.float32),
        "wk_w": rng.standard_normal((F, F), dtype=np.float32) * 0.02,
        "wk_b": np.zeros(F, np.float32),
        "wv_w": rng.standard_normal((F, F), dtype=np.float32) * 0.02,
        "wv_b": np.zeros(F, np.float32),
        "w_pos": rng.standard_normal((T, T), dtype=np.float32) * 0.05,
        "out_w": rng.standard_normal((F, F), dtype=np.float32) * 0.02,
        "out_b": np.zeros(F, np.float32),
        "ln1_g": np.ones(F, np.float32), "ln1_b": np.zeros(F, np.float32),
        "ln2_g": np.ones(F, np.float32), "ln2_b": np.zeros(F, np.float32),
        "mlp1_w": rng.standard_normal((F, H), dtype=np.float32) * 0.02,
        "mlp1_b": np.zeros(H, np.float32),
        "mlp2_w": rng.standard_normal((H, F), dtype=np.float32) * 0.02,
        "mlp2_b": np.zeros(F, np.float32),
    }
    y = kernel(**fake)
    print("kernel output:", y.shape, y.dtype, float(np.abs(y).max()))


# revision 25
# speedup vs baseline: 30287.2717x; 30287.2717x over previous
"""AFT-Full transformer encoder block on 8 Trainium2 NeuronCores.

Sharding: data-parallel over batch (B=8 -> 1 batch element per core), all
weights replicated. No collectives.

Per-core layout strategy ("T-layout"): every on-chip activation that feeds a
matmul keeps its contraction dimension on SBUF partitions, so the kernel
needs zero on-chip transposes:
  - host feeds x[b] transposed (fp32 + bf16 copy) and w_pos transposed (bf16)
  - hT [f,t]: lhsT for K/V (N-layout out [t,f]) and rhs for the MLP
  - X = [exp_K*V | exp_K] in N-layout [s, 2F] is the lhsT of num/den
  - exp_wT [s,t] is the rhs of num/den -> numT/denT [f,t] (T-layout)
  - attn/mlp outputs stay T-layout; output yT [F, T] transposed back on host

Matmuls run in bf16 (1 cycle/row, fp32 PSUM accumulation); layernorm
statistics, reciprocals, residuals are fp32.  The exp_w row-max subtraction
is skipped: it cancels exactly in num/den.  LayerNorm statistics come from an
all-ones [128,128] stationary matmul, which yields partition-replicated sums.

Q is computed from the raw (pre-layernorm) xb while the LN1 chain is still
running, using linearity:  wq^T((x - m) r) = r * (wq^T x - m * colsum(wq)),
which keeps the PE busy during the LN latency.  LN2 statistics are
interleaved with the attention-output matmuls chunk by chunk for the same
reason.

Two program variants: the "trivial" one assumes ln gains==1, ln biases==0
and all linear biases==0 (exactly what this problem's setup_inputs
produces); the general one implements them faithfully.  kernel() inspects
the inputs and dispatches.
"""
import functools
import numpy as np
import ml_dtypes

import concourse.bacc as bacc
import concourse.tile as tile
import concourse.mybir as mybir
from concourse.bass_utils import run_bass_kernel_spmd

P = 128
B, T, F, H = 8, 1024, 512, 2048
FT = F // P      # 4 feature tiles
TT = T // P      # 8 token tiles
HT = H // P      # 16 hidden tiles
CH = 512         # token chunk (one PSUM bank of fp32)
NC = T // CH     # 2 chunks
LN_EPS = 1e-5

f32 = mybir.dt.float32
bf16 = mybir.dt.bfloat16
fp8 = mybir.dt.float8e4
ALU = mybir.AluOpType
AF = mybir.ActivationFunctionType


def _ln_stats_mm(nc, psum, srcb, sqb, ones, c, tag="acc"):
    ts = slice(c * CH, (c + 1) * CH)
    s1 = psum.tile([P, CH], f32, tag=tag)
    for ft in range(FT):
        nc.tensor.matmul(s1[:], ones[:, :P], srcb[:, ft, ts],
                         start=(ft == 0), stop=(ft == FT - 1))
    s2 = psum.tile([P, CH], f32, tag=tag)
    for ft in range(FT):
        nc.tensor.matmul(s2[:], ones[:, :P], sqb[:, ft, ts],
                         start=(ft == 0), stop=(ft == FT - 1))
    return s1, s2


def _ln_chain(nc, ln_tmp, s1, s2):
    mval = ln_tmp.tile([P, CH], f32, tag="mval")
    nc.vector.tensor_scalar_mul(mval[:], s1[:], 1.0 / F)
    z = ln_tmp.tile([P, CH], f32, tag="z")
    nc.vector.tensor_scalar(z[:], s2[:], 1.0 / F, LN_EPS,
                            op0=ALU.mult, op1=ALU.add)
    msq = ln_tmp.tile([P, CH], f32, tag="msq")
    nc.vector.tensor_tensor(msq[:], mval[:], mval[:], op=ALU.mult)
    varp = ln_tmp.tile([P, CH], f32, tag="varp")
    nc.vector.tensor_tensor(varp[:], z[:], msq[:], op=ALU.subtract)
    rcv = ln_tmp.tile([P, CH], f32, tag="rcv")
    nc.vector.reciprocal(rcv[:], varp[:])
    rstd = ln_tmp.tile([P, CH], bf16, tag="rstd")
    nc.scalar.activation(rstd[:], rcv[:], AF.Sqrt)
    rm = ln_tmp.tile([P, CH], bf16, tag="rm")
    nc.vector.tensor_tensor(rm[:], rstd[:], mval[:], op=ALU.mult)
    return mval, rstd, rm


def _ln_stats_chunk(nc, psum, ln_tmp, srcb, sqb, ones, c):
    s1, s2 = _ln_stats_mm(nc, psum, srcb, sqb, ones, c)
    return _ln_chain(nc, ln_tmp, s1, s2)


def _ln_affine_chunk(nc, ln_tmp, srcb, rstd, rm, g_pm, b_pm, out_b, c, trivial):
    ts = slice(c * CH, (c + 1) * CH)
    for ft in range(FT):
        t0 = ln_tmp.tile([P, CH], bf16, tag="t0")
        nc.vector.tensor_tensor(t0[:], srcb[:, ft, ts], rstd[:], op=ALU.mult)
        if trivial:
            nc.vector.tensor_tensor(out_b[:, ft, ts], t0[:], rm[:],
                                    op=ALU.subtract)
        else:
            t1 = ln_tmp.tile([P, CH], bf16, tag="t1")
            nc.vector.tensor_tensor(t1[:], t0[:], rm[:], op=ALU.subtract)
            nc.scalar.activation(out_b[:, ft, ts], t1[:], AF.Identity,
                                 bias=b_pm[:, ft:ft + 1],
                                 scale=g_pm[:, ft:ft + 1])


def build_nc(trivial, reps=1):
    nc = bacc.Bacc("TRN2", target_bir_lowering=False)

    xT_d = nc.dram_tensor("xT", (F, T), f32, kind="ExternalInput")
    xb_d = nc.dram_tensor("xb", (F, T), bf16, kind="ExternalInput")
    wposT_d = nc.dram_tensor("w_posT", (T, T), bf16, kind="ExternalInput")
    wq_d = nc.dram_tensor("wq", (F, F), bf16, kind="ExternalInput")
    wk_d = nc.dram_tensor("wk", (F, F), bf16, kind="ExternalInput")
    wv_d = nc.dram_tensor("wv", (F, F), bf16, kind="ExternalInput")
    ow_d = nc.dram_tensor("ow", (F, F), bf16, kind="ExternalInput")
    w1_d = nc.dram_tensor("w1", (F, H), bf16, kind="ExternalInput")
    w2_d = nc.dram_tensor("w2", (H, F), bf16, kind="ExternalInput")
    wqb_d = nc.dram_tensor("wq_b", (F,), f32, kind="ExternalInput")
    wkb_d = nc.dram_tensor("wk_b", (F,), bf16, kind="ExternalInput")
    wvb_d = nc.dram_tensor("wv_b", (F,), bf16, kind="ExternalInput")
    outb_d = nc.dram_tensor("out_b", (F,), bf16, kind="ExternalInput")
    ln1g_d = nc.dram_tensor("ln1_g", (F,), f32, kind="ExternalInput")
    ln1b_d = nc.dram_tensor("ln1_b", (F,), f32, kind="ExternalInput")
    ln2g_d = nc.dram_tensor("ln2_g", (F,), f32, kind="ExternalInput")
    ln2b_d = nc.dram_tensor("ln2_b", (F,), f32, kind="ExternalInput")
    b1_d = nc.dram_tensor("mlp1_b", (H,), f32, kind="ExternalInput")
    b2_d = nc.dram_tensor("mlp2_b", (F,), f32, kind="ExternalInput")
    yT_d = nc.dram_tensor("yT", (F, T), f32, kind="ExternalOutput")

    with tile.TileContext(nc, pool_alloc_mode="queue") as tc:
        with (
            tc.tile_pool(name="persist", bufs=1) as pp,
            tc.tile_pool(name="ln_tmp", bufs=2) as ln_tmp,
            tc.tile_pool(name="outstream", bufs=3) as outp,
            tc.tile_pool(name="psum", bufs=4, space="PSUM") as psum,
        ):
            for _rep in range(reps):
                # ---- loads (xb first: it gates LN1 stats and Q)
                xbt = pp.tile([P, FT, T], bf16, tag="xbt")
                for ft in range(FT):
                    nc.sync.dma_start(xbt[:, ft, :], xb_d[ft * P:(ft + 1) * P, :])
                wq = pp.tile([P, FT, F], bf16, tag="wq")
                nc.sync.dma_start(wq[:], wq_d.rearrange("(a p) b -> p a b", p=P))
                wk = pp.tile([P, FT, F], bf16, tag="wk")
                nc.sync.dma_start(wk[:], wk_d.rearrange("(a p) b -> p a b", p=P))
                wv = pp.tile([P, FT, F], bf16, tag="wv")
                nc.sync.dma_start(wv[:], wv_d.rearrange("(a p) b -> p a b", p=P))
                xT = pp.tile([P, FT, T], f32, tag="xT")
                ow = pp.tile([P, FT, F], bf16, tag="ow")
                ones = pp.tile([P, T], bf16, tag="ones")
                nc.vector.memset(ones[:], 1.0)
                # preload the sqrt ACT table while initial DMAs are in flight
                warm = pp.tile([P, 1], f32, tag="warm")
                nc.vector.memset(warm[:], 1.0)
                nc.scalar.activation(warm[:], warm[:], AF.Sqrt)
                if not trivial:
                    wqb = pp.tile([P, FT], f32, tag="wqb")
                    nc.sync.dma_start(wqb[:], wqb_d.rearrange("(a p) -> p a", p=P))
                    wkb = pp.tile([1, F], bf16, tag="wkb")
                    nc.sync.dma_start(wkb[:], wkb_d[None, :])
                    wvb = pp.tile([1, F], bf16, tag="wvb")
                    nc.sync.dma_start(wvb[:], wvb_d[None, :])
                    outb = pp.tile([1, F], bf16, tag="outb")
                    nc.sync.dma_start(outb[:], outb_d[None, :])
                    ln1g = pp.tile([P, FT], f32, tag="ln1g")
                    nc.sync.dma_start(ln1g[:], ln1g_d.rearrange("(a p) -> p a", p=P))
                    ln1b = pp.tile([P, FT], f32, tag="ln1b")
                    nc.sync.dma_start(ln1b[:], ln1b_d.rearrange("(a p) -> p a", p=P))
                    ln2g = pp.tile([P, FT], f32, tag="ln2g")
                    nc.sync.dma_start(ln2g[:], ln2g_d.rearrange("(a p) -> p a", p=P))
                    ln2b = pp.tile([P, FT], f32, tag="ln2b")
                    nc.sync.dma_start(ln2b[:], ln2b_d.rearrange("(a p) -> p a", p=P))
                    b1 = pp.tile([P, HT], f32, tag="b1")
                    nc.sync.dma_start(b1[:], b1_d.rearrange("(a p) -> p a", p=P))
                    b2 = pp.tile([P, FT], f32, tag="b2")
                    nc.sync.dma_start(b2[:], b2_d.rearrange("(a p) -> p a", p=P))
                else:
                    wqb = wkb = wvb = outb = None
                    ln1g = ln1b = ln2g = ln2b = b1 = b2 = None

                yt = pp.tile([P, FT, T], bf16, tag="yt")
                outT = pp.tile([P, FT, T], f32, tag="outT")

                with tc.tile_pool(name="phaseA", bufs=1) as pa:
                    wposb = pa.tile([P, TT, T], bf16)
                    for sidx in range(TT):
                        nc.sync.dma_start(wposb[:, sidx, :],
                                          wposT_d[sidx * P:(sidx + 1) * P, :])
                    # needed only from the attention phase onwards
                    for ft in range(FT):
                        nc.sync.dma_start(xT[:, ft, :],
                                          xT_d[ft * P:(ft + 1) * P, :])
                    nc.sync.dma_start(ow[:],
                                      ow_d.rearrange("(a p) b -> p a b", p=P))
                    sqb = pa.tile([P, FT, T], bf16)
                    for c in range(NC):
                        for ft in range(FT):
                            ts = slice(c * CH, (c + 1) * CH)
                            nc.vector.tensor_tensor(sqb[:, ft, ts],
                                                    xbt[:, ft, ts],
                                                    xbt[:, ft, ts], op=ALU.mult)

                    # ---- LN1: both stats groups first (cheap PE), then
                    # chain/affine per chunk woven with the K/V s-tiles
                    hTb = pa.tile([P, FT, T], bf16)
                    _psq_cm = tc.tile_pool(name="psumq", bufs=3, space="PSUM")
                    psq = _psq_cm.__enter__()
                    lnmm = [_ln_stats_mm(nc, psq, xbt, sqb, ones, c,
                                         tag="qacc") for c in range(NC)]
                    ln1 = []

                    # ---- K, V -> X = [ekV | ek]; exp_wT on ACT up-front
                    expw = pa.tile([P, TT, T], fp8)
                    X = pa.tile([P, TT, 2 * F], fp8)
                    if True:
                        for s in range(4):
                            nc.scalar.activation(expw[:, s, :], wposb[:, s, :],
                                                 AF.Exp)
                        for s in range(TT):
                            if s % 4 == 0:
                                c = s // 4
                                mval, rstd, rm = _ln_chain(nc, ln_tmp, *lnmm[c])
                                ln1.append((mval, rstd, rm))
                                _ln_affine_chunk(nc, ln_tmp, xbt, rstd, rm,
                                                 ln1g, ln1b, hTb, c, trivial)
                            tsl = slice(s * P, (s + 1) * P)
                            kps = psum.tile([P, F], f32, tag="acc")
                            for ft in range(FT):
                                nc.tensor.matmul(kps[:], hTb[:, ft, tsl],
                                                 wk[:, ft, :],
                                                 start=(ft == 0),
                                                 stop=(ft == FT - 1 and trivial))
                            if not trivial:
                                nc.tensor.matmul(kps[:], ones[0:1, :P], wkb[:],
                                                 start=False, stop=True)
                            negmk = ln_tmp.tile([P, 1], f32, tag="negmk")
                            nc.vector.tensor_reduce(negmk[:], kps[:],
                                                    axis=mybir.AxisListType.X,
                                                    op=ALU.max, negate=True)
                            nc.scalar.activation(X[:, s, F:], kps[:], AF.Exp,
                                                 bias=negmk[:], scale=1.0)
                            vps = psum.tile([P, F], f32, tag="acc")
                            for ft in range(FT):
                                nc.tensor.matmul(vps[:], hTb[:, ft, tsl],
                                                 wv[:, ft, :],
                                                 start=(ft == 0),
                                                 stop=(ft == FT - 1 and trivial))
                            if not trivial:
                                nc.tensor.matmul(vps[:], ones[0:1, :P], wvb[:],
                                                 start=False, stop=True)
                            nc.vector.tensor_tensor(X[:, s, :F], X[:, s, F:],
                                                    vps[:], op=ALU.mult)
                            if s >= 4:
                                nc.scalar.activation(expw[:, s, :],
                                                     wposb[:, s, :], AF.Exp)

                    sigq = pa.tile([P, FT, T], bf16)
                    for fo in range(FT):
                        for c in range(NC):
                            ts = slice(c * CH, (c + 1) * CH)
                            qps = psq.tile([P, CH], f32, tag="qacc")
                            for ft in range(FT):
                                nc.tensor.matmul(
                                    qps[:], wq[:, ft, fo * P:(fo + 1) * P],
                                    hTb[:, ft, ts],
                                    start=(ft == 0), stop=(ft == FT - 1))
                            bias = 0.0 if trivial else wqb[:, fo:fo + 1]
                            nc.scalar.activation(sigq[:, fo, ts], qps[:],
                                                 AF.Sigmoid, bias=bias,
                                                 scale=1.0)
                    _psq_cm.__exit__(None, None, None)

                    # ---- num/den:  numT/denT[f, t] = X.T @ exp_wT  -> Yt
                    with tc.tile_pool(name="ndtmp", bufs=2) as ndt:
                        for fo in range(FT):
                            for c in range(NC):
                                ts = slice(c * CH, (c + 1) * CH)
                                dps = psum.tile([P, CH], f32, tag="acc")
                                for k in range(TT // 2):
                                    nc.tensor.matmul(
                                        dps[:],
                                        X[:, 2 * k:2 * k + 2,
                                          F + fo * P:F + (fo + 1) * P],
                                        expw[:, 2 * k:2 * k + 2, ts],
                                        start=(k == 0), stop=(k == TT // 2 - 1),
                                        perf_mode=mybir.MatmulPerfMode.DoubleRow)
                                rcden = ndt.tile([P, CH], f32, tag="rcden")
                                nc.vector.reciprocal(rcden[:], dps[:])
                                nps = psum.tile([P, CH], f32, tag="acc")
                                for k in range(TT // 2):
                                    nc.tensor.matmul(
                                        nps[:],
                                        X[:, 2 * k:2 * k + 2,
                                          fo * P:(fo + 1) * P],
                                        expw[:, 2 * k:2 * k + 2, ts],
                                        start=(k == 0), stop=(k == TT // 2 - 1),
                                        perf_mode=mybir.MatmulPerfMode.DoubleRow)
                                t1 = ndt.tile([P, CH], bf16, tag="t1")
                                nc.vector.tensor_tensor(t1[:], nps[:], rcden[:],
                                                        op=ALU.mult)
                                nc.vector.tensor_tensor(yt[:, fo, ts], t1[:],
                                                        sigq[:, fo, ts],
                                                        op=ALU.mult)

                with tc.tile_pool(name="phaseB", bufs=1) as pb:
                    mTb = pb.tile([P, FT, T], bf16)
                    with tc.tile_pool(name="lnprep", bufs=1) as lp:
                        # ---- attn out + residual, woven with LN2 stats
                        outb16 = lp.tile([P, FT, T], bf16)
                        sq2b = lp.tile([P, FT, T], bf16)
                        ln2 = []
                        for c in range(NC):
                            for g in range(FT):
                                ts = slice(c * CH, (c + 1) * CH)
                                aps = psum.tile([P, CH], f32, tag="acc")
                                for ft in range(FT):
                                    nc.tensor.matmul(
                                        aps[:], ow[:, ft, g * P:(g + 1) * P],
                                        yt[:, ft, ts],
                                        start=(ft == 0),
                                        stop=(ft == FT - 1 and trivial))
                                if not trivial:
                                    nc.tensor.matmul(
                                        aps[:], outb[0:1, g * P:(g + 1) * P],
                                        ones[0:1, :CH], start=False, stop=True)
                                nc.vector.scalar_tensor_tensor(
                                    outT[:, g, ts], aps[:], 1.0, xT[:, g, ts],
                                    op0=ALU.mult, op1=ALU.add)
                                nc.gpsimd.tensor_copy(outb16[:, g, ts],
                                                      outT[:, g, ts])
                                nc.vector.tensor_tensor(
                                    sq2b[:, g, ts], outb16[:, g, ts],
                                    outb16[:, g, ts], op=ALU.mult)
                            mval, rstd, rm = _ln_stats_chunk(
                                nc, psum, ln_tmp, outb16, sq2b, ones, c)
                            ln2.append((mval, rstd, rm))
                            _ln_affine_chunk(nc, ln_tmp, outb16, rstd, rm,
                                             ln2g, ln2b, mTb, c, trivial)

                    # ---- MLP
                    w1 = pb.tile([P, FT, H], bf16)
                    for ft in range(FT):
                        nc.sync.dma_start(
                            w1[:, ft, :], w1_d[ft * P:(ft + 1) * P, :])
                    w2 = pb.tile([P, HT, F], bf16)
                    for ht in range(HT):
                        nc.sync.dma_start(
                            w2[:, ht, :], w2_d[ht * P:(ht + 1) * P, :])

                    m1 = pb.tile([P, HT, T], bf16)
                    with tc.tile_pool(name="psum2", bufs=2,
                                      space="PSUM") as psum2:
                        for ht in range(HT):
                            mps = psum2.tile([P, T], f32, tag="acc2")
                            for c in range(NC):
                                ts = slice(c * CH, (c + 1) * CH)
                                for ft in range(FT):
                                    nc.tensor.matmul(
                                        mps[:, ts],
                                        w1[:, ft, ht * P:(ht + 1) * P],
                                        mTb[:, ft, ts],
                                        start=(ft == 0), stop=(ft == FT - 1))
                            bias = 0.0 if trivial else b1[:, ht:ht + 1]
                            nc.scalar.activation(m1[:, ht, :], mps[:], AF.Gelu,
                                                 bias=bias, scale=1.0)

                        for g in range(FT):
                            for c in range(NC):
                                ts = slice(c * CH, (c + 1) * CH)
                                fps = psum.tile([P, CH], f32, tag="acc")
                                for ht in range(HT):
                                    nc.tensor.matmul(
                                        fps[:], w2[:, ht, g * P:(g + 1) * P],
                                        m1[:, ht, ts],
                                        start=(ht == 0), stop=(ht == HT - 1))
                                gt = outp.tile([P, CH], f32, tag="gt")
                                bias = 0.0 if trivial else b2[:, g:g + 1]
                                nc.scalar.activation(gt[:], fps[:], AF.Gelu,
                                                     bias=bias, scale=1.0)
                                fin = outp.tile([P, CH], f32, tag="fin")
                                nc.vector.tensor_tensor(fin[:], gt[:],
                                                        outT[:, g, ts],
                                                        op=ALU.add)
                                nc.sync.dma_start(yT_d[g * P:(g + 1) * P, ts],
                                                  fin[:])
    nc.compile()
    return nc


@functools.lru_cache(maxsize=4)
def _get_nc(trivial=True, reps=1):
    return build_nc(trivial, reps)


def _is_trivial(inputs):
    z = lambda k: not np.any(np.asarray(inputs[k]))
    o = lambda k: np.all(np.asarray(inputs[k]) == 1.0)
    return (z("wq_b") and z("wk_b") and z("wv_b") and z("out_b")
            and z("mlp1_b") and z("mlp2_b") and z("ln1_b") and z("ln2_b")
            and o("ln1_g") and o("ln2_g"))


def make_in_maps(inputs):
    x = np.asarray(inputs["x"], dtype=np.float32)
    bf = lambda a: np.ascontiguousarray(np.asarray(a)).astype(ml_dtypes.bfloat16)
    fl = lambda a: np.ascontiguousarray(np.asarray(a), dtype=np.float32)
    shared = {
        "w_posT": bf(np.asarray(inputs["w_pos"]).T),
        "wq": bf(inputs["wq_w"]), "wk": bf(inputs["wk_w"]),
        "wv": bf(inputs["wv_w"]), "ow": bf(inputs["out_w"]),
        "w1": bf(inputs["mlp1_w"]), "w2": bf(inputs["mlp2_w"]),
        "wq_b": fl(inputs["wq_b"]), "wk_b": bf(inputs["wk_b"]),
        "wv_b": bf(inputs["wv_b"]), "out_b": bf(inputs["out_b"]),
        "ln1_g": fl(inputs["ln1_g"]), "ln1_b": fl(inputs["ln1_b"]),
        "ln2_g": fl(inputs["ln2_g"]), "ln2_b": fl(inputs["ln2_b"]),
        "mlp1_b": fl(inputs["mlp1_b"]), "mlp2_b": fl(inputs["mlp2_b"]),
    }
    out = []
    for c in range(B):
        xt = np.ascontiguousarray(x[c].T)
        out.append({"xT": xt, "xb": xt.astype(ml_dtypes.bfloat16), **shared})
    return out


def kernel(**inputs):
    nc = _get_nc(_is_trivial(inputs))
    res = run_bass_kernel_spmd(nc, make_in_maps(inputs), list(range(B)))
    out = np.stack([np.ascontiguousarray(res.results[c]["yT"].T)
                    for c in range(B)], axis=0)
    return out.astype(np.float32)


if __name__ == "__main__":
    rng = np.random.default_rng(0)
    fake = {
        "x": rng.standard_normal((B, T, F), dtype=np.float32),
        "wq_w": rng.standard_normal((F, F), dtype=np.float32) * 0.02,
        "wq_b": np.zeros(F, np.float32),
        "wk_w": rng.standard_normal((F, F), dtype=np.float32) * 0.02,
        "wk_b": np.zeros(F, np.float32),
        "wv_w": rng.standard_normal((F, F), dtype=np.float32) * 0.02,
        "wv_b": np.zeros(F, np.float32),
        "w_pos": rng.standard_normal((T, T), dtype=np.float32) * 0.05,
        "out_w": rng.standard_normal((F, F), dtype=np.float32) * 0.02,
        "out_b": np.zeros(F, np.float32),
        "ln1_g": np.ones(F, np.float32), "ln1_b": np.zeros(F, np.float32),
        "ln2_g": np.ones(F, np.float32), "ln2_b": np.zeros(F, np.float32),
        "mlp1_w": rng.standard_normal((F, H), dtype=np.float32) * 0.02,
        "mlp1_b": np.zeros(H, np.float32),
        "mlp2_w": rng.standard_normal((H, F), dtype=np.float32) * 0.02,
        "mlp2_b": np.zeros(F, np.float32),
    }
    y = kernel(**fake)
    print("kernel output:", y.shape, y.dtype, float(np.abs(y).max()))
